# revision 1
# baseline (speedup 1.0000x reference)
"""Trainium2 Bass kernel for a 3-head GCN block (graph conv + 1x1 conv + BN + residual ReLU).

Computes, for x:(N,C,T,V)=(128,64,128,25), A:(3,25,25), Wd:(3,64,64):
    out = relu(BN(sum_h Wd_h @ (A_h-mix along V of x)) + x)

Strategy (data-parallel over batch N across 8 cores, NS=16 each):
  * Load x with (2 batches x 64 channels) in SBUF partitions, (t, v) in free dim
    (the only DMA-efficient layout: 12.8KB contiguous runs).
  * Conv stage (contract C=64) on the PE:
      lhsT = x-chunk (64c, TSZ*25=(t,u)), rhs = WdTaug (64c, 256)
      where WdTaug columns = [Wd_0^T | Wd_1^T | Wd_2^T | diag(1/bn_scale)].
      The 4th block routes the residual x through the same pipeline pre-divided
      by the BN scale so the final epilogue multiply restores plain x.
      Output lands *already transposed* as zT (t,u partitions, head/ch free).
  * Graph stage (contract V=25) in bf16 with block-diagonal constant rhs
      BD_h[(i,u),(j,v)] = delta_ij * A[h,v,u], grp 3 = identity.
      lhsT = zT-slice (TSZ*25, 128=2n*64o) -> PSUM output is directly in the
      DMA-friendly (n,o partitions; t,v free) layout. 4 accumulating matmuls.
  * Epilogue: single ScalarE activation out = relu(scale*psum + shift) with
    per-partition BN vectors; fat 1.6MB DMAs in/out per 2-batch tile.
No transposes, no collectives.

MODE="fp32r": conv in float32r (full-rate fp32; needs even free-AP counts ->
  TSZ=4, M=100) . MODE="bf16": conv in bf16 (x cast on DVE), TSZ=5, M=125.
"""

import os

import numpy as np
import ml_dtypes

import concourse.bass as bass
import concourse.tile as tile
from concourse import bacc, mybir
from concourse import bass_utils

BN_EPS = 1e-5

# Problem shapes (hardcoded per contract)
N, C, T, V, H = 128, 64, 128, 25, 3
NCORES = 8
NS = N // NCORES  # 16 batches per core
NPAIRS = NS // 2  # 8 two-batch tiles per core

MODE = os.environ.get("GCN_MODE", "fp32r")  # "fp32r" | "bf16"

_CACHE = {}


def _build_nc(mode):
    """Build the per-core Bass program (identical on all 8 cores)."""
    f32 = mybir.dt.float32
    f32r = mybir.dt.float32r
    bf16 = mybir.dt.bfloat16

    tsz = 4 if mode == "fp32r" else 5  # t-values per chunk
    M = tsz * V  # zT partitions per chunk (100 or 125)
    if T % tsz == 0:
        chunks = [(i * tsz, tsz) for i in range(T // tsz)]
    else:
        nfull = T // tsz
        chunks = [(i * tsz, tsz) for i in range(nfull)] + [(nfull * tsz, T % tsz)]
    xdt = f32r if mode == "fp32r" else bf16

    nc = bacc.Bacc("TRN2", target_bir_lowering=False, debug=False)

    x_d = nc.dram_tensor(
        "x", (NS, C, T, V), f32r if mode == "fp32r" else f32, kind="ExternalInput"
    ).ap()
    wdt_d = nc.dram_tensor("wdt", (128, 2, 256), xdt, kind="ExternalInput").ap()
    bd_d = nc.dram_tensor("bd", (M, 4, M), bf16, kind="ExternalInput").ap()
    sc_d = nc.dram_tensor("sc", (128, 1), f32, kind="ExternalInput").ap()
    sh_d = nc.dram_tensor("sh", (128, 1), f32, kind="ExternalInput").ap()
    out_d = nc.dram_tensor("out", (NS, C, T, V), f32, kind="ExternalOutput").ap()

    with tile.TileContext(nc) as tc:
        with (
            tc.tile_pool(name="consts", bufs=1) as consts,
            tc.tile_pool(name="xo", bufs=3) as xo,
            tc.tile_pool(name="zt", bufs=4) as ztp,
            tc.tile_pool(name="ps_zt", bufs=2, space="PSUM") as ps_zt,
            tc.tile_pool(name="ps_g", bufs=2, space="PSUM") as ps_g,
        ):
            wdt_sb = consts.tile([128, 2, 256], xdt)
            nc.sync.dma_start(out=wdt_sb[:], in_=wdt_d[:])
            bd_sb = consts.tile([M, 4, M], bf16)
            nc.sync.dma_start(out=bd_sb[:], in_=bd_d[:])
            sc_sb = consts.tile([128, 1], f32)
            nc.sync.dma_start(out=sc_sb[:], in_=sc_d[:])
            sh_sb = consts.tile([128, 1], f32)
            nc.sync.dma_start(out=sh_sb[:], in_=sh_d[:])

            relu = mybir.ActivationFunctionType.Relu

            for p in range(NPAIRS):
                if mode == "fp32r":
                    x_tile = xo.tile([128, T, V], f32r, tag="x")
                    nc.sync.dma_start(
                        out=x_tile[:],
                        in_=x_d[2 * p : 2 * p + 2].rearrange("a c t v -> (a c) t v"),
                    )
                    x_mm = x_tile
                else:
                    x_tile = xo.tile([128, T, V], f32, tag="x")
                    nc.sync.dma_start(
                        out=x_tile[:],
                        in_=x_d[2 * p : 2 * p + 2].rearrange("a c t v -> (a c) t v"),
                    )
                    x_mm = xo.tile([128, T, V], bf16, tag="xb")
                    nc.vector.tensor_copy(x_mm[:], x_tile[:])
                out_tile = xo.tile([128, T, V], f32, tag="o")

                for ci, (t0, tc_sz) in enumerate(chunks):
                    Mc = tc_sz * V
                    # conv: one K=128 matmul, block-diag rhs covers both batches
                    zt_ps = ps_zt.tile([Mc, 2, 4, 64], f32, tag="zt_ps")
                    nc.tensor.matmul(
                        zt_ps[:],
                        lhsT=x_mm[:, t0 : t0 + tc_sz, :],
                        rhs=wdt_sb[:],
                        start=True,
                        stop=True,
                    )
                    # PSUM -> SBUF copy with bf16 cast, (half,grp) -> (grp,half)
                    zt_sb = ztp.tile([Mc, 4, 2, 64], bf16, tag="zt_sb")
                    zt_out_ap = zt_sb.rearrange("m g h o -> m h g o")
                    if ci % 2 == 0:
                        nc.vector.tensor_copy(zt_out_ap, zt_ps[:])
                    else:
                        nc.scalar.copy(zt_out_ap, zt_ps[:])
                    # graph stage: 4 accumulating matmuls (3 heads + residual)
                    g_ps = ps_g.tile([128, tc_sz, V], f32, tag="g_ps")
                    for grp in range(4):
                        nc.tensor.matmul(
                            g_ps[:],
                            lhsT=zt_sb[:, grp],
                            rhs=bd_sb[:Mc, grp, :Mc],
                            start=(grp == 0),
                            stop=(grp == 3),
                        )
                    # epilogue: relu(scale * g + shift)
                    nc.scalar.activation(
                        out_tile[:, t0 : t0 + tc_sz, :],
                        g_ps[:],
                        relu,
                        bias=sh_sb[:],
                        scale=sc_sb[:],
                    )

                nc.sync.dma_start(
                    out=out_d[2 * p : 2 * p + 2].rearrange("a c t v -> (a c) t v"),
                    in_=out_tile[:],
                )

    nc.compile()
    return nc, tsz


def _get_nc(mode=None):
    mode = mode or MODE
    key = f"nc_{mode}"
    if key not in _CACHE:
        _CACHE[key] = _build_nc(mode)
    return _CACHE[key]


def _host_consts(A, Wd, bd, gamma, beta, run_mean, run_var, mode, tsz):
    """Tiny host-side constant preprocessing."""
    A = np.asarray(A, np.float32)
    Wd = np.asarray(Wd, np.float32)
    bd = np.asarray(bd, np.float32)
    gamma = np.asarray(gamma, np.float32)
    beta = np.asarray(beta, np.float32)
    run_mean = np.asarray(run_mean, np.float32)
    run_var = np.asarray(run_var, np.float32)

    scale = gamma / np.sqrt(run_var + BN_EPS)  # (64,)
    shift = (bd.sum(axis=0) - run_mean) * scale + beta  # (64,)

    wdt = np.zeros((64, 256), np.float32)
    for h in range(H):
        wdt[:, h * 64 : (h + 1) * 64] = Wd[h].T  # [c, o] = Wd[h, o, c]
    wdt[:, 192:256] = np.diag(1.0 / scale)
    # block-diagonal over the two batches sharing the 128 partitions
    wdt2 = np.zeros((128, 2, 256), np.float32)
    wdt2[0:64, 0] = wdt
    wdt2[64:128, 1] = wdt
    if mode == "bf16":
        wdt2 = wdt2.astype(ml_dtypes.bfloat16)

    M = tsz * V
    bdm = np.zeros((M, 4, M), np.float32)
    for h in range(H):
        for i in range(tsz):
            bdm[i * 25 : (i + 1) * 25, h, i * 25 : (i + 1) * 25] = A[h].T
    bdm[:, 3, :] = np.eye(M, dtype=np.float32)
    bdm = bdm.astype(ml_dtypes.bfloat16)

    sc2 = np.tile(scale, 2)[:, None].astype(np.float32)  # (128,1)
    sh2 = np.tile(shift, 2)[:, None].astype(np.float32)
    return wdt2, bdm, sc2, sh2


def _in_maps(x, A, Wd, bd, gamma, beta, run_mean, run_var, mode=None, tsz=None):
    mode = mode or MODE
    if tsz is None:
        tsz = 4 if mode == "fp32r" else 5
    x = np.ascontiguousarray(np.asarray(x, np.float32))
    wdt2, bdm, sc2, sh2 = _host_consts(
        A, Wd, bd, gamma, beta, run_mean, run_var, mode, tsz
    )
    return [
        {
            "x": x[i * NS : (i + 1) * NS],
            "wdt": wdt2,
            "bd": bdm,
            "sc": sc2,
            "sh": sh2,
        }
        for i in range(NCORES)
    ]


def kernel(x, A, Wd, bd, gamma, beta, run_mean, run_var, _trace=False):
    nc, tsz = _get_nc()
    in_maps = _in_maps(x, A, Wd, bd, gamma, beta, run_mean, run_var, MODE, tsz)
    res = bass_utils.run_bass_kernel_spmd(
        nc, in_maps, core_ids=list(range(NCORES)), trace=_trace
    )
    out = np.concatenate(
        [np.asarray(r["out"], np.float32) for r in res.results], axis=0
    )
    _CACHE["last_results"] = res
    return out

